# revision 8
# baseline (speedup 1.0000x reference)
"""BiLSTM-CRF NLL kernel for 8 Trainium2 NeuronCores.

Strategy (3 SPMD launches, host glue between them):
  L1 "layer0": 8 cores = 2 dirs x 4 batch-quarters (16 seqs/core, one LSTM dir).
     Per core the 16 sequences run as TWO staggered chains of 8 so the
     recurrent serial path (matmul block -> sigmoid -> DVE cell update ->
     tanh -> h-mult) of one chain hides under the other chain's engine work.
     gx = W_ih @ x (+bias via a ones-plane) is accumulated directly into a
     per-chunk PSUM bank; the per-step W_hh @ h matmuls accumulate into the
     same bank (no identity-prefill matmuls). One merged sigmoid covers all
     8 gate tiles per step. Cell state c2 = 2*c lives in PSUM so the tanh
     reads through the fast ScalarE<->PSUM port.
  L2 "layer1": same program shape with K=512 input; host reshards and handles
     the per-sequence reversal of the backward direction.
  L3 "logits+CRF": 8 cores = 8 batch-eighths. Logits matmul, then the CRF
     partition function as an exp-domain matrix recursion
     a_t = (E^T a_{t-1}) * exp(logit_t - 3), E = exp(trans) on the PE, with
     periodic per-seq renormalization (the -3 shift keeps fp32 in range so
     renorm only every 16 steps; host adds 3*len back to the partition).
     Masking is avoided by keeping the whole a_t history and extracting
     column t=len_b-1 per sequence via a host-built one-hot mask.
     start/end/transition numerator terms are summed on host.

Matmuls run in bf16 (fp32 PSUM accumulate); cell state c and CRF are fp32.
"""

import os
import sys

import numpy as np

for _p in ("/opt/trn_rl_repo", "/root/.axon_site/_ro/trn_rl_repo"):
    if _p not in sys.path and os.path.isdir(_p):
        sys.path.insert(0, _p)

import ml_dtypes  # noqa: E402

BF16 = ml_dtypes.bfloat16

B, T, V, E, HD, NT = 64, 256, 50000, 256, 256, 20
NCORES = 8
BL = 16            # sequences per core in L1/L2 (batch quarter)
NCH_G = 2          # chains per core in L1/L2
BL2 = BL // NCH_G  # sequences per chain (8)
BC = 8             # sequences per core in L3 (batch eighth)
NJ = 8             # gate tiles of 128 rows (4 gates x 256 HD / 128)
TCH = 8            # timesteps per gx chunk (one PSUM bank per chain-chunk)
NCHUNKS = T // TCH
NCH3 = 512         # matmul N-chunk in the CRF logits prologue
RENORM_EVERY = 16  # CRF renormalization interval (exp(-3) shift keeps range)
CRF_SHIFT = 3.0    # logit shift inside the CRF exp
NREN = (T - 1) // RENORM_EVERY   # renorm slots used
NTOK3 = BC * T

# gate order stays pytorch-native (i,f,g,o): tiles [i0 i1 f0 f1 g0 g1 o0 o1].
# g rows are pre-scaled by 2 so tanh(x) = 2*sig(2x)-1.
_PERM = np.arange(4 * HD)

_CACHE = {}
LAST_RESULTS = []   # BassKernelResults of the launches of the last kernel() call

H_ON_GPS = bool(int(os.environ.get("KERNEL_H_GPS", "1")))


def _mods():
    import concourse.bass as bass
    import concourse.tile as tile
    from concourse import bacc, mybir
    from concourse.bass_utils import run_bass_kernel_spmd
    return bass, tile, bacc, mybir, run_bass_kernel_spmd


def _install_ntff_shim():
    """Provide antenv.axon_hooks (missing in this image) so that
    run_bass_kernel_spmd(trace=True) can capture NTFF profiles through
    libaxon_pjrt.so. Mirrors trn_agent_boot._ntff_profile_via_ctypes."""
    import sys as _sys
    if "antenv.axon_hooks" in _sys.modules:
        return
    import contextlib
    import ctypes
    import types

    so_path = "/opt/axon/libaxon_pjrt.so"
    mod = types.ModuleType("antenv.axon_hooks")
    _hook_box = [None]

    def set_axon_ntff_profile_hook(h):
        _hook_box[0] = h

    def get_axon_ntff_profile_hook():
        return _hook_box[0]

    mod.set_axon_ntff_profile_hook = set_axon_ntff_profile_hook
    mod.get_axon_ntff_profile_hook = get_axon_ntff_profile_hook
    _sys.modules["antenv.axon_hooks"] = mod

    try:
        lib = ctypes.CDLL(so_path)
        if not hasattr(lib, "axon_start_nrt_profile"):
            return
        lib.axon_start_nrt_profile.argtypes = [
            ctypes.POINTER(ctypes.c_int64), ctypes.c_size_t]
        lib.axon_start_nrt_profile.restype = ctypes.c_int64
        lib.axon_stop_nrt_profile.argtypes = [ctypes.c_char_p]
        lib.axon_stop_nrt_profile.restype = ctypes.c_int64

        @contextlib.contextmanager
        def _hook(output_dir, device_ids):
            import jax
            jax.devices()
            if device_ids:
                ids = (ctypes.c_int64 * len(device_ids))(*device_ids)
                rc = lib.axon_start_nrt_profile(ids, len(device_ids))
            else:
                rc = lib.axon_start_nrt_profile(None, 0)
            if rc != 0:
                raise RuntimeError(f"axon_start_nrt_profile rc={rc}")
            try:
                yield
            finally:
                n = lib.axon_stop_nrt_profile(str(output_dir).encode())
                print(f"profile: {n} file(s) written to {output_dir}",
                      file=sys.stderr)

        set_axon_ntff_profile_hook(_hook)
    except OSError:
        pass


# --------------------------------------------------------------------------
# program builders
# --------------------------------------------------------------------------

def build_layer_program(kc_in):
    """One BiLSTM direction for BL sequences as 2 staggered chains of BL2.

    kc_in = input dim / 128 (data planes; one extra ones-plane adds bias).
    """
    bass, tile, bacc, mybir, _ = _mods()
    dt = mybir.dt
    AF = mybir.ActivationFunctionType
    AO = mybir.AluOpType

    NPL = kc_in + 1  # input planes incl. bias plane

    nc = bacc.Bacc("TRN2", target_bir_lowering=False, debug=False)
    xT = nc.dram_tensor("xT", [NPL, 128, T, BL], dt.bfloat16,
                        kind="ExternalInput").ap()
    wih = nc.dram_tensor("wih", [NPL, 128, 4 * HD], dt.bfloat16,
                         kind="ExternalInput").ap()
    whh = nc.dram_tensor("whh", [2, 128, 4 * HD], dt.bfloat16,
                         kind="ExternalInput").ap()
    hout = nc.dram_tensor("hout", [128, 2, T, BL], dt.bfloat16,
                          kind="ExternalOutput").ap()

    with tile.TileContext(nc) as tc:
        with (
            tc.tile_pool(name="w", bufs=1) as wpool,
            tc.tile_pool(name="big", bufs=1) as big,
            tc.tile_pool(name="xs0", bufs=3) as xs0,
            tc.tile_pool(name="xs1", bufs=3) as xs1,
            tc.tile_pool(name="ew0", bufs=2) as ew0,
            tc.tile_pool(name="ew1", bufs=2) as ew1,
            tc.tile_pool(name="gx0", bufs=2, space="PSUM") as gx0,
            tc.tile_pool(name="gx1", bufs=2, space="PSUM") as gx1,
            tc.tile_pool(name="cst0", bufs=1, space="PSUM") as cst0,
            tc.tile_pool(name="cst1", bufs=1, space="PSUM") as cst1,
        ):
            wih_sb = wpool.tile([128, NPL, 4 * HD], dt.bfloat16)
            whh_sb = wpool.tile([128, 2, 4 * HD], dt.bfloat16)
            for pl in range(NPL):
                nc.sync.dma_start(wih_sb[:, pl, :], wih[pl])
            for kc in range(2):
                nc.sync.dma_start(whh_sb[:, kc, :], whh[kc])

            xpools = (xs0, xs1)
            gxpools = (gx0, gx1)
            ewpools = (ew0, ew1)
            cstpools = (cst0, cst1)

            hists = []
            c2s = []
            for g in range(NCH_G):
                hist = big.tile([128, 2, T + 1, BL2], dt.bfloat16,
                                name=f"hist{g}")
                nc.vector.memset(hist[:, :, 0, :], 0.0)
                hists.append(hist)
                # per-chain PSUM bank: ScalarE tanh reads PSUM fast, and the
                # chains' concurrent ScE-read/VecE-write stay in distinct banks
                c2 = cstpools[g].tile([128, 2, BL2], dt.float32,
                                      name=f"c2_{g}")
                nc.vector.memset(c2[:], 0.0)
                c2s.append(c2)

            def load_x(g, n):
                xc = xpools[g].tile([128, NPL, TCH, BL2], dt.bfloat16,
                                    name=f"xc{g}")
                for pl in range(NPL):
                    nc.sync.dma_start(
                        xc[:, pl, :, :],
                        xT[pl, :, n * TCH:(n + 1) * TCH,
                           g * BL2:(g + 1) * BL2])
                return xc

            def new_gx(g):
                # one PSUM bank: [gate tile j, step-in-chunk, seq]
                return gxpools[g].tile([128, NJ, TCH, BL2], dt.float32,
                                       name=f"gx{g}")

            def gx_mm(gxb, xc, j, pl):
                # accumulate W_ih[j-tile] @ x(plane) into the chunk bank.
                # start=True only on the chunk's very first matmul: it clears
                # the whole bank's has_written bits; every later matmul
                # (including the per-step W_hh ones) accumulates/overwrites
                # per element. stop stays open until the chunk's last W_hh mm.
                nc.tensor.matmul(
                    gxb[:, j, :, :], wih_sb[:, pl, j * 128:(j + 1) * 128],
                    xc[:, pl, :, :].rearrange("p t b -> p (t b)"),
                    start=(j == 0 and pl == 0), stop=False,
                    skip_group_check=True)

            # gx matmul issue schedule: spread the NPL*NJ chunk matmuls of
            # chunk n+1 across the TCH steps of chunk n.
            gx_jobs = [(j, pl) for j in range(NJ) for pl in range(NPL)]
            per_step = (len(gx_jobs) + TCH - 1) // TCH
            gx_sched = [gx_jobs[i * per_step:(i + 1) * per_step]
                        for i in range(TCH)]

            def step(g, n, tt, gxb, gxn, xcn):
                t = n * TCH + tt
                hist, c2 = hists[g], c2s[g]
                # W_hh @ h_{t-1} accumulated onto gx(+bias) in PSUM
                for j in range(NJ):
                    for kc in range(2):
                        nc.tensor.matmul(
                            gxb[:, j, tt, :],
                            whh_sb[:, kc, j * 128:(j + 1) * 128],
                            hist[:, kc, t, :],
                            start=False,
                            stop=(tt == TCH - 1 and j == NJ - 1 and kc == 1),
                            skip_group_check=True)
                # interleave next chunk's gx matmuls into this step's slot
                if gxn is not None:
                    for (j, pl) in gx_sched[tt]:
                        gx_mm(gxn, xcn, j, pl)
                # merged sigmoid over all 8 gate tiles (PSUM -> SBUF)
                A = ewpools[g].tile([128, NJ, BL2], dt.float32, name=f"A{g}")
                nc.scalar.activation(A[:], gxb[:, :, tt, :], AF.Sigmoid)
                # u = (sig(2g) - 0.5) * sig(i)  [= tanh(g)*sig(i)/2]
                u = ewpools[g].tile([128, 2, BL2], dt.float32, name=f"u{g}")
                nc.vector.scalar_tensor_tensor(
                    u[:], A[:, 4:6, :], 0.5, A[:, 0:2, :],
                    AO.subtract, AO.mult)
                # v = sig(f) * c2_{t-1}
                v = ewpools[g].tile([128, 2, BL2], dt.float32, name=f"v{g}")
                nc.vector.tensor_tensor(v[:], A[:, 2:4, :], c2[:], AO.mult)
                # c2_t = 4u + v   (c2 = 2c)
                nc.vector.scalar_tensor_tensor(
                    c2[:], u[:], 4.0, v[:], AO.mult, AO.add)
                # tanh(c) = tanh(0.5 * c2), fast ScalarE PSUM read
                Tc = ewpools[g].tile([128, 2, BL2], dt.float32, name=f"T{g}")
                nc.scalar.activation(Tc[:], c2[:], AF.Tanh, scale=0.5)
                # h_t = sig(o) * tanh(c) -> bf16 history
                eng = nc.gpsimd if H_ON_GPS else nc.vector
                eng.tensor_tensor(hist[:, :, t + 1, :], A[:, 6:8, :], Tc[:],
                                  AO.mult)

            # software pipeline (per chain):
            #   gxcur[g] = complete gx chunk being consumed (chunk n)
            #   gxnxt[g] = chunk n+1's bank, filled by jobs during chunk n
            #   xnxt[g]  = x data those jobs read (chunk n+1)
            x0 = [load_x(g, 0) for g in range(NCH_G)]
            gxcur = []
            for g in range(NCH_G):
                gxb = new_gx(g)
                for (j, pl) in gx_jobs:
                    gx_mm(gxb, x0[g], j, pl)
                gxcur.append(gxb)
            xnxt = [load_x(g, 1) for g in range(NCH_G)]
            gxnxt = [new_gx(g) for g in range(NCH_G)]

            for n in range(NCHUNKS):
                xfut = None
                if n + 2 < NCHUNKS:
                    xfut = [load_x(g, n + 2) for g in range(NCH_G)]
                for tt in range(TCH):
                    for g in range(NCH_G):
                        step(g, n, tt, gxcur[g],
                             gxnxt[g] if gxnxt is not None else None,
                             xnxt[g] if xnxt is not None else None)
                # stream finished history out
                t0 = n * TCH
                for g in range(NCH_G):
                    for kc in range(2):
                        nc.sync.dma_start(
                            hout[:, kc, t0:t0 + TCH, g * BL2:(g + 1) * BL2],
                            hists[g][:, kc, t0 + 1:t0 + TCH + 1, :])
                # rotate the pipeline
                if n + 1 < NCHUNKS:
                    gxcur = gxnxt
                    xnxt = xfut
                    gxnxt = ([new_gx(g) for g in range(NCH_G)]
                             if n + 2 < NCHUNKS else None)
    nc.compile()
    return nc


def build_crf_program():
    bass, tile, bacc, mybir, _ = _mods()
    dt = mybir.dt
    AF = mybir.ActivationFunctionType
    AO = mybir.AluOpType

    nc = bacc.Bacc("TRN2", target_bir_lowering=False, debug=False)
    hcat = nc.dram_tensor("hcat", [4, 128, NTOK3], dt.bfloat16, kind="ExternalInput").ap()
    linw = nc.dram_tensor("linw", [4, 128, NT], dt.bfloat16, kind="ExternalInput").ap()
    linb = nc.dram_tensor("linb", [NT, 1], dt.float32, kind="ExternalInput").ap()
    etrans = nc.dram_tensor("etrans", [NT, NT], dt.float32, kind="ExternalInput").ap()
    estart = nc.dram_tensor("estart", [NT, 1], dt.float32, kind="ExternalInput").ap()
    eend = nc.dram_tensor("eend", [NT, 1], dt.float32, kind="ExternalInput").ap()
    emitmask = nc.dram_tensor("emitmask", [NT, NTOK3], dt.bfloat16, kind="ExternalInput").ap()
    lastsel = nc.dram_tensor("lastsel", [NT, BC, T], dt.bfloat16, kind="ExternalInput").ap()
    smask = nc.dram_tensor("smask", [1, BC, NREN + 1], dt.float32, kind="ExternalInput").ap()
    part_out = nc.dram_tensor("part_out", [1, BC], dt.float32, kind="ExternalOutput").ap()
    emit_out = nc.dram_tensor("emit_out", [1, 1], dt.float32, kind="ExternalOutput").ap()

    NCHUNKS3 = NTOK3 // NCH3  # 4

    with tile.TileContext(nc) as tc:
        with (
            tc.tile_pool(name="w", bufs=1) as wpool,
            tc.tile_pool(name="big", bufs=1) as big,
            tc.tile_pool(name="sm", bufs=4) as sm,
            tc.tile_pool(name="pslg", bufs=2, space="PSUM") as pslg,
            tc.tile_pool(name="ps", bufs=2, space="PSUM") as ps,
        ):
            hc_sb = big.tile([128, 4, NTOK3], dt.bfloat16)
            for kc in range(4):
                nc.sync.dma_start(hc_sb[:, kc, :], hcat[kc])
            lw_sb = wpool.tile([128, 4, NT], dt.bfloat16)
            for kc in range(4):
                nc.sync.dma_start(lw_sb[:, kc, :], linw[kc])
            lb_sb = wpool.tile([NT, 1], dt.float32)
            nc.sync.dma_start(lb_sb[:], linb[:])
            et_sb = wpool.tile([NT, NT], dt.float32)
            nc.sync.dma_start(et_sb[:], etrans[:])
            es_sb = wpool.tile([NT, 1], dt.float32)
            nc.sync.dma_start(es_sb[:], estart[:])
            ee_sb = wpool.tile([NT, 1], dt.float32)
            nc.sync.dma_start(ee_sb[:], eend[:])
            em_sb = big.tile([NT, NTOK3], dt.bfloat16)
            nc.sync.dma_start(em_sb[:], emitmask[:])
            ls_sb = big.tile([NT, BC, T], dt.bfloat16)
            nc.sync.dma_start(ls_sb[:], lastsel[:])
            sm_sb = wpool.tile([1, BC, NREN + 1], dt.float32)
            nc.sync.dma_start(sm_sb[:], smask[:])
            ones_sb = wpool.tile([NT, 1], dt.float32)
            nc.vector.memset(ones_sb[:], 1.0)
            onesrow = wpool.tile([1, NT], dt.float32)
            nc.vector.memset(onesrow[:], 1.0)
            nshift = wpool.tile([NT, 1], dt.float32)
            nc.vector.memset(nshift[:], -CRF_SHIFT)

            # logits^T [NT, t, b] fp32, and exp(logits - CRF_SHIFT)
            logits = big.tile([NT, T, BC], dt.float32)
            for n in range(NCHUNKS3):
                acc = pslg.tile([NT, NCH3], dt.float32, name="lg")
                for kc in range(4):
                    nc.tensor.matmul(acc[:], lw_sb[:, kc, :],
                                     hc_sb[:, kc, n * NCH3:(n + 1) * NCH3],
                                     start=(kc == 0), stop=(kc == 3))
                accv = acc[:].rearrange("p (t b) -> p t b", b=BC)
                nc.vector.tensor_scalar_add(
                    logits[:, n * (NCH3 // BC):(n + 1) * (NCH3 // BC), :],
                    accv, lb_sb[:])
            elog = big.tile([NT, T, BC], dt.float32)
            nc.scalar.activation(elog[:], logits[:], AF.Exp, bias=nshift[:])

            # exp-domain forward recursion, two chains of 4 sequences
            NBH = BC // 2
            shist = big.tile([1, BC, NREN + 1], dt.float32)
            nc.vector.memset(shist[:], 1.0)
            ahists = []
            for c in range(2):
                ah = big.tile([NT, NBH, T], dt.float32, name=f"ah{c}")
                nc.vector.tensor_scalar_mul(
                    ah[:, :, 0], elog[:, 0, c * NBH:(c + 1) * NBH], es_sb[:])
                ahists.append(ah)
            for t in range(1, T):
                for c in range(2):
                    ah = ahists[c]
                    bsl = slice(c * NBH, (c + 1) * NBH)
                    y = ps.tile([NT, NBH], dt.float32, name=f"y{c}")
                    nc.tensor.matmul(y[:], et_sb[:], ah[:, :, t - 1],
                                     start=True, stop=True)
                    if t % RENORM_EVERY == 0:
                        r = t // RENORM_EVERY - 1
                        ssum = ps.tile([NT, NBH], dt.float32, name="aux", bufs=1)[0:1]
                        nc.tensor.matmul(ssum[:], ones_sb[:], ah[:, :, t - 1],
                                         start=True, stop=True)
                        nc.vector.tensor_copy(shist[:, bsl, r], ssum[:])
                        rinv = sm.tile([1, NBH], dt.float32, name=f"rinv{c}")
                        nc.vector.reciprocal(rinv[:], ssum[:])
                        rb = ps.tile([NT, NBH], dt.float32, name="aux", bufs=1)
                        nc.tensor.matmul(rb[:], onesrow[:], rinv[:],
                                         start=True, stop=True)
                        u1 = sm.tile([NT, NBH], dt.float32, name=f"u1{c}")
                        nc.vector.tensor_tensor(u1[:], y[:], elog[:, t, bsl],
                                                AO.mult)
                        nc.vector.tensor_tensor(ah[:, :, t], u1[:], rb[:],
                                                AO.mult)
                    else:
                        nc.vector.tensor_tensor(ah[:, :, t], y[:],
                                                elog[:, t, bsl], AO.mult)

            # partition_b = ln(sum_j a[len_b-1, j] * e_end[j]) + sum_r ln(s_rb)
            # (host adds CRF_SHIFT * len_b back)
            alast = sm.tile([NT, BC], dt.float32)
            for c in range(2):
                bsl = slice(c * NBH, (c + 1) * NBH)
                prod = big.tile([NT, NBH, T], dt.float32, name=f"prod{c}")
                nc.vector.tensor_tensor(prod[:], ahists[c][:], ls_sb[:, bsl, :],
                                        AO.mult)
                nc.vector.reduce_sum(alast[:, bsl], prod[:],
                                     axis=mybir.AxisListType.X)
            w2 = sm.tile([NT, BC], dt.float32)
            nc.vector.tensor_scalar_mul(w2[:], alast[:], ee_sb[:])
            fsum = ps.tile([1, BC], dt.float32, name="faux", bufs=1)
            nc.tensor.matmul(fsum[:], ones_sb[:], w2[:], start=True, stop=True)
            pln = sm.tile([1, BC], dt.float32)
            nc.scalar.activation(pln[:], fsum[:], AF.Ln)
            slog = sm.tile([1, BC, NREN + 1], dt.float32)
            nc.scalar.activation(slog[:], shist[:], AF.Ln)
            slogm = sm.tile([1, BC, NREN + 1], dt.float32)
            nc.vector.tensor_tensor(slogm[:], slog[:], sm_sb[:], AO.mult)
            zb = sm.tile([1, BC], dt.float32)
            nc.vector.reduce_sum(zb[:], slogm[:], axis=mybir.AxisListType.X)
            pout = sm.tile([1, BC], dt.float32)
            nc.vector.tensor_tensor(pout[:], pln[:], zb[:], AO.add)
            nc.sync.dma_start(part_out[:], pout[:])

            # emission score total
            eprod = big.tile([NT, T, BC], dt.float32)
            nc.vector.tensor_tensor(
                eprod[:], logits[:],
                em_sb[:].rearrange("p (t b) -> p t b", b=BC), AO.mult)
            erow = sm.tile([NT, 1], dt.float32)
            nc.vector.reduce_sum(erow[:], eprod[:], axis=mybir.AxisListType.XY)
            etot = ps.tile([1, 1], dt.float32, name="faux", bufs=1)
            nc.tensor.matmul(etot[:], ones_sb[:], erow[:], start=True, stop=True)
            eout = sm.tile([1, 1], dt.float32)
            nc.vector.tensor_copy(eout[:], etot[:])
            nc.sync.dma_start(emit_out[:], eout[:])
    nc.compile()
    return nc


# --------------------------------------------------------------------------
# host-side data prep
# --------------------------------------------------------------------------

def _layer_inputs(xin, w_ih, w_hh, b_ih, b_hh):
    """Per-core input dicts for one layer launch.

    xin: [2, B, T, K] fp32 (xin[1] already reversed+masked)
    w_ih: [2, 4HD, K]; w_hh: [2, 4HD, HD]; b_ih, b_hh: [2, 4HD]
    """
    K = xin.shape[-1]
    kc_in = K // 128
    # scale the g-gate rows (block 3) by 2: tanh(x) = 2*sig(2x)-1
    gscale = np.ones((4 * HD, 1), np.float32)
    gscale[2 * HD:3 * HD] = 2.0
    per_dir = []
    for d in range(2):
        wih_p = w_ih[d][_PERM] * gscale
        whh_p = w_hh[d][_PERM] * gscale
        b_p = (b_ih[d] + b_hh[d])[_PERM] * gscale[:, 0]
        wihT = np.zeros((kc_in + 1, 128, 4 * HD), np.float32)
        wihT[:kc_in] = wih_p.T.reshape(kc_in, 128, 4 * HD)
        wihT[kc_in, 0, :] = b_p          # bias plane: row 0 only
        whhT = np.ascontiguousarray(
            whh_p.T.reshape(2, 128, 4 * HD)).astype(BF16)
        per_dir.append((wihT.astype(BF16), whhT))
    maps = []
    for core in range(NCORES):
        d, q = divmod(core, 4)
        xc = xin[d, q * BL:(q + 1) * BL]              # [BL, T, K]
        xT = np.zeros((kc_in + 1, 128, T, BL), np.float32)
        xT[:kc_in] = xc.transpose(2, 1, 0).reshape(kc_in, 128, T, BL)
        xT[kc_in, 0] = 1.0               # ones plane: row 0 only
        wihT, whhT = per_dir[d]
        maps.append({"xT": np.ascontiguousarray(xT).astype(BF16),
                     "wih": wihT, "whh": whhT})
    return maps


def _collect_h(results):
    """per-core 'hout' [128,2,T,BL] bf16 -> h [2, B, T, HD] fp32."""
    h = np.empty((2, B, T, HD), np.float32)
    for core in range(NCORES):
        d, q = divmod(core, 4)
        ho = np.asarray(results[core]["hout"], dtype=np.float32)
        ho = ho.reshape(128, 2, T, BL)
        h[d, q * BL:(q + 1) * BL] = ho.transpose(3, 2, 1, 0).reshape(BL, T, HD)
    return h


def _unreverse(h_rev, lens, valid):
    """h_rev[b, s] holds position lens_b-1-s; return h[b, t] (zeros at pad)."""
    t = np.arange(T)
    idx = np.clip(lens[:, None] - 1 - t[None, :], 0, T - 1)
    out = np.take_along_axis(h_rev, idx[:, :, None], axis=1)
    return out * valid[:, :, None]


def kernel(**inputs):
    _, _, _, _, run_bass_kernel_spmd = _mods()
    global LAST_RESULTS
    LAST_RESULTS = []
    trace = bool(int(os.environ.get("KERNEL_TRACE", "0")))
    if trace:
        _install_ntff_shim()

    tokens = np.asarray(inputs["tokens"]).astype(np.int64)
    lens = np.asarray(inputs["lens"]).astype(np.int64)
    labels = np.asarray(inputs["labels"]).astype(np.int64)
    emb = np.asarray(inputs["emb"], dtype=np.float32)
    w_ih = [np.asarray(inputs["w_ih_l0"], np.float32),
            np.asarray(inputs["w_ih_l1"], np.float32)]
    w_hh = [np.asarray(inputs["w_hh_l0"], np.float32),
            np.asarray(inputs["w_hh_l1"], np.float32)]
    b_ih = [np.asarray(inputs["b_ih_l0"], np.float32),
            np.asarray(inputs["b_ih_l1"], np.float32)]
    b_hh = [np.asarray(inputs["b_hh_l0"], np.float32),
            np.asarray(inputs["b_hh_l1"], np.float32)]
    lin_w = np.asarray(inputs["lin_w"], np.float32)
    lin_b = np.asarray(inputs["lin_b"], np.float32)
    trans = np.asarray(inputs["trans"], np.float32)
    start_t = np.asarray(inputs["start_t"], np.float32)
    end_t = np.asarray(inputs["end_t"], np.float32)

    t_ar = np.arange(T)
    valid = (t_ar[None, :] < lens[:, None]).astype(np.float32)
    rev_idx = np.clip(lens[:, None] - 1 - t_ar[None, :], 0, T - 1)

    if "layer0" not in _CACHE:
        _CACHE["layer0"] = build_layer_program(E // 128)
    if "layer1" not in _CACHE:
        _CACHE["layer1"] = build_layer_program(2 * HD // 128)
    if "crf" not in _CACHE:
        _CACHE["crf"] = build_crf_program()

    cores = list(range(NCORES))

    # ---------- launch 1: layer 0 ----------
    x = emb[tokens]
    x_rev = np.take_along_axis(x, rev_idx[:, :, None], axis=1) * valid[:, :, None]
    xin0 = np.stack([x, x_rev])
    res1 = run_bass_kernel_spmd(
        _CACHE["layer0"], _layer_inputs(xin0, w_ih[0], w_hh[0], b_ih[0], b_hh[0]),
        cores, trace=trace)
    LAST_RESULTS.append(res1)
    h0 = _collect_h(res1.results)

    # ---------- launch 2: layer 1 ----------
    h0f = h0[0] * valid[:, :, None]
    h0b = _unreverse(h0[1], lens, valid)
    x1 = np.concatenate([h0f, h0b], axis=-1)
    x1_rev = np.take_along_axis(x1, rev_idx[:, :, None], axis=1) * valid[:, :, None]
    xin1 = np.stack([x1, x1_rev])
    res2 = run_bass_kernel_spmd(
        _CACHE["layer1"], _layer_inputs(xin1, w_ih[1], w_hh[1], b_ih[1], b_hh[1]),
        cores, trace=trace)
    LAST_RESULTS.append(res2)
    h1 = _collect_h(res2.results)

    # ---------- launch 3: logits + CRF ----------
    h1f = h1[0] * valid[:, :, None]
    h1b = _unreverse(h1[1], lens, valid)
    hcat = np.concatenate([h1f, h1b], axis=-1)

    lw = np.ascontiguousarray(lin_w.T.reshape(4, 128, NT)).astype(BF16)
    et = np.exp(trans).astype(np.float32)
    es = np.exp(start_t).astype(np.float32)[:, None]
    ee = np.exp(end_t).astype(np.float32)[:, None]
    lb = np.ascontiguousarray(lin_b.astype(np.float32)[:, None])
    maps = []
    for core in range(NCORES):
        bs = slice(core * BC, (core + 1) * BC)
        hc = hcat[bs]
        hcT = np.ascontiguousarray(
            hc.transpose(2, 1, 0).reshape(4, 128, T * BC)).astype(BF16)
        em = np.zeros((NT, T, BC), np.float32)
        lab = labels[bs]
        for bb in range(BC):
            em[lab[bb], np.arange(T), bb] = valid[bs][bb]
        ls = np.zeros((NT, BC, T), np.float32)
        for bb in range(BC):
            ls[:, bb, lens[bs][bb] - 1] = 1.0
        r_idx = np.arange(NREN + 1)
        smk = (RENORM_EVERY * (r_idx[None] + 1)
               <= (lens[bs] - 1)[:, None]).astype(np.float32)[None]
        maps.append({
            "hcat": hcT, "linw": lw, "linb": lb, "etrans": et,
            "estart": es, "eend": ee,
            "emitmask": np.ascontiguousarray(
                em.reshape(NT, T * BC)).astype(BF16),
            "lastsel": np.ascontiguousarray(ls).astype(BF16),
            "smask": np.ascontiguousarray(smk),
        })
    res3 = run_bass_kernel_spmd(_CACHE["crf"], maps, cores, trace=trace)
    LAST_RESULTS.append(res3)

    partition = np.concatenate(
        [np.asarray(r["part_out"])[0] for r in res3.results])
    # undo the exp(-CRF_SHIFT) per-step shift: a'_t = a_t * e^{-shift*(t+1)}
    partition = partition + CRF_SHIFT * lens.astype(np.float32)
    emit = float(sum(np.asarray(r["emit_out"])[0, 0] for r in res3.results))

    # host-side numerator terms
    first_tag = labels[:, 0]
    last_tag = np.take_along_axis(labels, (lens - 1)[:, None], axis=1)[:, 0]
    tr_sc = float((trans[labels[:, :-1], labels[:, 1:]] * valid[:, 1:]).sum())
    host_num = float(start_t[first_tag].sum()) + tr_sc + float(end_t[last_tag].sum())

    loss = partition.sum() - emit - host_num
    return np.float32(loss)


# revision 14
# speedup vs baseline: 1.6161x; 1.6161x over previous
"""BiLSTM-CRF NLL kernel for 8 Trainium2 NeuronCores.

Strategy (3 SPMD launches, host glue between them):
  L1 "layer0": 8 cores = 2 dirs x 4 batch-quarters (16 seqs/core, one LSTM dir).
     Per core the 16 sequences run as TWO staggered chains of 8 so the
     recurrent serial path (matmul block -> sigmoid -> DVE cell update ->
     tanh -> h-mult) of one chain hides under the other chain's engine work.
     gx = W_ih @ x (+bias via a ones-plane) is accumulated directly into a
     per-chunk PSUM bank; the per-step W_hh @ h matmuls accumulate into the
     same bank (no identity-prefill matmuls). One merged sigmoid covers all
     8 gate tiles per step. Cell state c2 = 2*c lives in PSUM so the tanh
     reads through the fast ScalarE<->PSUM port.
  L2 "layer1": same program shape with K=512 input; host reshards and handles
     the per-sequence reversal of the backward direction.
  L3 "logits+CRF": 8 cores = 8 batch-eighths. Logits matmul, then the CRF
     partition function as an exp-domain matrix recursion
     a_t = (E^T a_{t-1}) * exp(logit_t - 3), E = exp(trans) on the PE, with
     periodic per-seq renormalization (the -3 shift keeps fp32 in range so
     renorm only every 16 steps; host adds 3*len back to the partition).
     Masking is avoided by keeping the whole a_t history and extracting
     column t=len_b-1 per sequence via a host-built one-hot mask.
     start/end/transition numerator terms are summed on host.

Matmuls run in bf16 (fp32 PSUM accumulate); cell state c and CRF are fp32.
"""

import os
import sys

import numpy as np

for _p in ("/opt/trn_rl_repo", "/root/.axon_site/_ro/trn_rl_repo"):
    if _p not in sys.path and os.path.isdir(_p):
        sys.path.insert(0, _p)

import ml_dtypes  # noqa: E402

BF16 = ml_dtypes.bfloat16

B, T, V, E, HD, NT = 64, 256, 50000, 256, 256, 20
NCORES = 8
BL = 16            # sequences per core in L1/L2 (batch quarter)
NCH_G = 1          # chains per core in L1/L2
BL2 = BL // NCH_G  # sequences per chain (8)
BC = 8             # sequences per core in L3 (batch eighth)
NJ = 8             # gate tiles of 128 rows (4 gates x 256 HD / 128)
TCH = 8            # timesteps per gx chunk (one PSUM bank per chain-chunk)
NCHUNKS = T // TCH
NCH3 = 512         # matmul N-chunk in the CRF logits prologue
RENORM_EVERY = 16  # CRF renormalization interval (exp(-3) shift keeps range)
CRF_SHIFT = 3.0    # logit shift inside the CRF exp
NREN = (T - 1) // RENORM_EVERY   # renorm slots used
NTOK3 = BC * T

# gate tiles reordered [f0 f1 i0 i1 g0 g1 o0 o1] so the f-gate matmuls and
# sigmoid issue first (v = sig(f)*c2 overlaps the i/g sigmoid).
# g rows are pre-scaled by 2 so tanh(x) = 2*sig(2x)-1.
_PERM = np.concatenate([np.arange(HD, 2 * HD), np.arange(0, HD),
                        np.arange(2 * HD, 3 * HD), np.arange(3 * HD, 4 * HD)])

_CACHE = {}
LAST_RESULTS = []   # BassKernelResults of the launches of the last kernel() call

H_ON_GPS = bool(int(os.environ.get("KERNEL_H_GPS", "0")))


def _mods():
    import concourse.bass as bass
    import concourse.tile as tile
    from concourse import bacc, mybir
    from concourse.bass_utils import run_bass_kernel_spmd
    return bass, tile, bacc, mybir, run_bass_kernel_spmd


def _install_ntff_shim():
    """Provide antenv.axon_hooks (missing in this image) so that
    run_bass_kernel_spmd(trace=True) can capture NTFF profiles through
    libaxon_pjrt.so. Mirrors trn_agent_boot._ntff_profile_via_ctypes."""
    import sys as _sys
    if "antenv.axon_hooks" in _sys.modules:
        return
    import contextlib
    import ctypes
    import types

    so_path = "/opt/axon/libaxon_pjrt.so"
    mod = types.ModuleType("antenv.axon_hooks")
    _hook_box = [None]

    def set_axon_ntff_profile_hook(h):
        _hook_box[0] = h

    def get_axon_ntff_profile_hook():
        return _hook_box[0]

    mod.set_axon_ntff_profile_hook = set_axon_ntff_profile_hook
    mod.get_axon_ntff_profile_hook = get_axon_ntff_profile_hook
    _sys.modules["antenv.axon_hooks"] = mod

    try:
        lib = ctypes.CDLL(so_path)
        if not hasattr(lib, "axon_start_nrt_profile"):
            return
        lib.axon_start_nrt_profile.argtypes = [
            ctypes.POINTER(ctypes.c_int64), ctypes.c_size_t]
        lib.axon_start_nrt_profile.restype = ctypes.c_int64
        lib.axon_stop_nrt_profile.argtypes = [ctypes.c_char_p]
        lib.axon_stop_nrt_profile.restype = ctypes.c_int64

        @contextlib.contextmanager
        def _hook(output_dir, device_ids):
            import jax
            jax.devices()
            if device_ids:
                ids = (ctypes.c_int64 * len(device_ids))(*device_ids)
                rc = lib.axon_start_nrt_profile(ids, len(device_ids))
            else:
                rc = lib.axon_start_nrt_profile(None, 0)
            if rc != 0:
                raise RuntimeError(f"axon_start_nrt_profile rc={rc}")
            try:
                yield
            finally:
                n = lib.axon_stop_nrt_profile(str(output_dir).encode())
                print(f"profile: {n} file(s) written to {output_dir}",
                      file=sys.stderr)

        set_axon_ntff_profile_hook(_hook)
    except OSError:
        pass


# --------------------------------------------------------------------------
# program builders
# --------------------------------------------------------------------------

def build_layer_program(kc_in):
    """One BiLSTM direction for BL sequences as 2 staggered chains of BL2.

    kc_in = input dim / 128 (data planes; one extra ones-plane adds bias).
    """
    bass, tile, bacc, mybir, _ = _mods()
    dt = mybir.dt
    AF = mybir.ActivationFunctionType
    AO = mybir.AluOpType

    NPL = kc_in + 1  # input planes incl. bias plane

    nc = bacc.Bacc("TRN2", target_bir_lowering=False, debug=False)
    xT = nc.dram_tensor("xT", [NPL, 128, T, BL], dt.bfloat16,
                        kind="ExternalInput").ap()
    wih = nc.dram_tensor("wih", [NPL, 128, 4 * HD], dt.bfloat16,
                         kind="ExternalInput").ap()
    whh = nc.dram_tensor("whh", [2, 128, 4 * HD], dt.bfloat16,
                         kind="ExternalInput").ap()
    hout = nc.dram_tensor("hout", [128, 2, T, BL], dt.bfloat16,
                          kind="ExternalOutput").ap()

    with tile.TileContext(nc) as tc:
        with (
            tc.tile_pool(name="w", bufs=1) as wpool,
            tc.tile_pool(name="big", bufs=1) as big,
            tc.tile_pool(name="xs0", bufs=3) as xs0,
            tc.tile_pool(name="xs1", bufs=3) as xs1,
            tc.tile_pool(name="ew0", bufs=2) as ew0,
            tc.tile_pool(name="ew1", bufs=2) as ew1,
            tc.tile_pool(name="gx0", bufs=2, space="PSUM") as gx0,
            tc.tile_pool(name="gx1", bufs=2, space="PSUM") as gx1,
            tc.tile_pool(name="cst0", bufs=1, space="PSUM") as cst0,
            tc.tile_pool(name="cst1", bufs=1, space="PSUM") as cst1,
        ):
            wih_sb = wpool.tile([128, NPL, 4 * HD], dt.bfloat16)
            whh_sb = wpool.tile([128, 2, 4 * HD], dt.bfloat16)
            for pl in range(NPL):
                nc.sync.dma_start(wih_sb[:, pl, :], wih[pl])
            for kc in range(2):
                nc.sync.dma_start(whh_sb[:, kc, :], whh[kc])

            xpools = (xs0, xs1)
            gxpools = (gx0, gx1)
            ewpools = (ew0, ew1)
            cstpools = (cst0, cst1)

            hists = []
            c2s = []
            for g in range(NCH_G):
                hist = big.tile([128, 2, T + 1, BL2], dt.bfloat16,
                                name=f"hist{g}")
                nc.vector.memset(hist[:, :, 0, :], 0.0)
                hists.append(hist)
                # per-chain PSUM bank: ScalarE tanh reads PSUM fast, and the
                # chains' concurrent ScE-read/VecE-write stay in distinct banks
                c2 = cstpools[g].tile([128, 2, BL2], dt.float32,
                                      name=f"c2_{g}")
                nc.vector.memset(c2[:], 0.0)
                c2s.append(c2)

            def load_x(g, n):
                xc = xpools[g].tile([128, NPL, TCH, BL2], dt.bfloat16,
                                    name=f"xc{g}")
                for pl in range(NPL):
                    nc.sync.dma_start(
                        xc[:, pl, :, :],
                        xT[pl, :, n * TCH:(n + 1) * TCH,
                           g * BL2:(g + 1) * BL2])
                return xc

            # gate tile j -> (stage bank, index within bank): the chunk is
            # split into three single-bank PSUM tiles by pipeline stage
            # (f / i+g / o) so a sigmoid reading one stage's bank never
            # blocks the next stage's matmul writes (PSUM deps are
            # bank-granular). Step-major layout inside each bank keeps the
            # sigmoid reads contiguous.
            _GRP = {0: (0, 0), 1: (0, 1), 2: (1, 0), 3: (1, 1),
                    4: (1, 2), 5: (1, 3), 6: (2, 0), 7: (2, 1)}
            _GRPW = (2, 4, 2)

            def new_gx(g):
                return tuple(
                    gxpools[g].tile([128, TCH, _GRPW[s], BL2], dt.float32,
                                    name=f"gx{g}s{s}")
                    for s in range(3))

            def gx_mm(gxt, xc, j, pl):
                # accumulate W_ih[j-tile] @ x(plane) into the chunk banks.
                # start=True on each bank's first matmul (clears the bank's
                # has_written bits); everything later (including the per-step
                # W_hh matmuls) accumulates per element. The group stays open
                # until the chunk's last W_hh matmul in that bank.
                s, jj = _GRP[j]
                nc.tensor.matmul(
                    gxt[s][:, :, jj, :], wih_sb[:, pl, j * 128:(j + 1) * 128],
                    xc[:, pl, :, :].rearrange("p t b -> p (t b)"),
                    start=(j in (0, 2, 6) and pl == 0), stop=False,
                    skip_group_check=True)

            # gx matmul issue schedule: spread the NPL*NJ chunk matmuls of
            # chunk n+1 across the TCH steps of chunk n.
            gx_jobs = [(j, pl) for j in range(NJ) for pl in range(NPL)]
            per_step = (len(gx_jobs) + TCH - 1) // TCH
            gx_sched = [gx_jobs[i * per_step:(i + 1) * per_step]
                        for i in range(TCH)]

            def step(g, n, tt, gxt, gxn, xcn):
                t = n * TCH + tt
                hist, c2 = hists[g], c2s[g]

                def whh_mm(j):
                    s, jj = _GRP[j]
                    # per-bank group close at the chunk's last W_hh matmul
                    for kc in range(2):
                        nc.tensor.matmul(
                            gxt[s][:, tt, jj, :],
                            whh_sb[:, kc, j * 128:(j + 1) * 128],
                            hist[:, kc, t, :],
                            start=False,
                            stop=(tt == TCH - 1 and j in (1, 5, 7)
                                  and kc == 1),
                            skip_group_check=True)

                # gate tiles are [f f i i g g o o]; f first so sig(f) and
                # v = sig(f)*c2 run while the i/g matmuls + sigmoid proceed
                A = ewpools[g].tile([128, NJ, BL2], dt.float32, name=f"A{g}")
                for j in (0, 1):
                    whh_mm(j)
                nc.scalar.activation(A[:, 0:2, :], gxt[0][:, tt, :, :],
                                     AF.Sigmoid)
                for j in (2, 3, 4, 5):
                    whh_mm(j)
                nc.scalar.activation(A[:, 2:6, :], gxt[1][:, tt, :, :],
                                     AF.Sigmoid)
                # v = sig(f) * c2_{t-1}  (first in the DVE queue)
                v = ewpools[g].tile([128, 2, BL2], dt.float32, name=f"v{g}")
                nc.vector.tensor_tensor(v[:], A[:, 0:2, :], c2[:], AO.mult)
                for j in (6, 7):
                    whh_mm(j)
                nc.scalar.activation(A[:, 6:8, :], gxt[2][:, tt, :, :],
                                     AF.Sigmoid)
                # u = (sig(2g) - 0.5) * sig(i)  [= tanh(g)*sig(i)/2]
                u = ewpools[g].tile([128, 2, BL2], dt.float32, name=f"u{g}")
                nc.vector.scalar_tensor_tensor(
                    u[:], A[:, 4:6, :], 0.5, A[:, 2:4, :],
                    AO.subtract, AO.mult)
                # c2_t = 4u + v   (c2 = 2c)
                nc.vector.scalar_tensor_tensor(
                    c2[:], u[:], 4.0, v[:], AO.mult, AO.add)
                # interleave next chunk's gx matmuls behind this step's mms
                if gxn is not None:
                    for (j, pl) in gx_sched[tt]:
                        gx_mm(gxn, xcn, j, pl)
                # tanh(c) = tanh(0.5 * c2), fast ScalarE PSUM read
                Tc = ewpools[g].tile([128, 2, BL2], dt.float32, name=f"T{g}")
                nc.scalar.activation(Tc[:], c2[:], AF.Tanh, scale=0.5)
                # h_t = sig(o) * tanh(c) -> bf16 history
                eng = nc.gpsimd if H_ON_GPS else nc.vector
                eng.tensor_tensor(hist[:, :, t + 1, :], A[:, 6:8, :], Tc[:],
                                  AO.mult)

            # software pipeline (per chain):
            #   gxcur[g] = complete gx chunk being consumed (chunk n)
            #   gxnxt[g] = chunk n+1's bank, filled by jobs during chunk n
            #   xnxt[g]  = x data those jobs read (chunk n+1)
            x0 = [load_x(g, 0) for g in range(NCH_G)]
            gxcur = []
            for g in range(NCH_G):
                gxb = new_gx(g)
                for (j, pl) in gx_jobs:
                    gx_mm(gxb, x0[g], j, pl)
                gxcur.append(gxb)
            xnxt = [load_x(g, 1) for g in range(NCH_G)]
            gxnxt = [new_gx(g) for g in range(NCH_G)]

            for n in range(NCHUNKS):
                xfut = None
                if n + 2 < NCHUNKS:
                    xfut = [load_x(g, n + 2) for g in range(NCH_G)]
                for tt in range(TCH):
                    for g in range(NCH_G):
                        step(g, n, tt, gxcur[g],
                             gxnxt[g] if gxnxt is not None else None,
                             xnxt[g] if xnxt is not None else None)
                # stream finished history out
                t0 = n * TCH
                for g in range(NCH_G):
                    for kc in range(2):
                        nc.sync.dma_start(
                            hout[:, kc, t0:t0 + TCH, g * BL2:(g + 1) * BL2],
                            hists[g][:, kc, t0 + 1:t0 + TCH + 1, :])
                # rotate the pipeline
                if n + 1 < NCHUNKS:
                    gxcur = gxnxt
                    xnxt = xfut
                    gxnxt = ([new_gx(g) for g in range(NCH_G)]
                             if n + 2 < NCHUNKS else None)
    nc.compile()
    return nc


def build_crf_program():
    bass, tile, bacc, mybir, _ = _mods()
    dt = mybir.dt
    AF = mybir.ActivationFunctionType
    AO = mybir.AluOpType

    nc = bacc.Bacc("TRN2", target_bir_lowering=False, debug=False)
    hcat = nc.dram_tensor("hcat", [4, 128, NTOK3], dt.bfloat16, kind="ExternalInput").ap()
    linw = nc.dram_tensor("linw", [4, 128, NT], dt.bfloat16, kind="ExternalInput").ap()
    linb = nc.dram_tensor("linb", [NT, 1], dt.float32, kind="ExternalInput").ap()
    etrans = nc.dram_tensor("etrans", [NT, NT], dt.float32, kind="ExternalInput").ap()
    estart = nc.dram_tensor("estart", [NT, 1], dt.float32, kind="ExternalInput").ap()
    eend = nc.dram_tensor("eend", [NT, 1], dt.float32, kind="ExternalInput").ap()
    emitmask = nc.dram_tensor("emitmask", [NT, NTOK3], dt.bfloat16, kind="ExternalInput").ap()
    lastsel = nc.dram_tensor("lastsel", [NT, T, BC], dt.bfloat16, kind="ExternalInput").ap()
    smask = nc.dram_tensor("smask", [1, BC, NREN + 1], dt.float32, kind="ExternalInput").ap()
    part_out = nc.dram_tensor("part_out", [1, BC], dt.float32, kind="ExternalOutput").ap()
    emit_out = nc.dram_tensor("emit_out", [1, 1], dt.float32, kind="ExternalOutput").ap()

    NCHUNKS3 = NTOK3 // NCH3  # 4

    with tile.TileContext(nc) as tc:
        with (
            tc.tile_pool(name="w", bufs=1) as wpool,
            tc.tile_pool(name="big", bufs=1) as big,
            tc.tile_pool(name="sm", bufs=4) as sm,
            tc.tile_pool(name="pslg", bufs=2, space="PSUM") as pslg,
            tc.tile_pool(name="ps", bufs=2, space="PSUM") as ps,
        ):
            hc_sb = big.tile([128, 4, NTOK3], dt.bfloat16)
            for kc in range(4):
                nc.sync.dma_start(hc_sb[:, kc, :], hcat[kc])
            lw_sb = wpool.tile([128, 4, NT], dt.bfloat16)
            for kc in range(4):
                nc.sync.dma_start(lw_sb[:, kc, :], linw[kc])
            lb_sb = wpool.tile([NT, 1], dt.float32)
            nc.sync.dma_start(lb_sb[:], linb[:])
            et_sb = wpool.tile([NT, NT], dt.float32)
            nc.sync.dma_start(et_sb[:], etrans[:])
            es_sb = wpool.tile([NT, 1], dt.float32)
            nc.sync.dma_start(es_sb[:], estart[:])
            ee_sb = wpool.tile([NT, 1], dt.float32)
            nc.sync.dma_start(ee_sb[:], eend[:])
            em_sb = big.tile([NT, NTOK3], dt.bfloat16)
            nc.sync.dma_start(em_sb[:], emitmask[:])
            ls_sb = big.tile([NT, T, BC], dt.bfloat16)
            nc.sync.dma_start(ls_sb[:], lastsel[:])
            sm_sb = wpool.tile([1, BC, NREN + 1], dt.float32)
            nc.sync.dma_start(sm_sb[:], smask[:])
            ones_sb = wpool.tile([NT, 1], dt.float32)
            nc.vector.memset(ones_sb[:], 1.0)
            onesrow = wpool.tile([1, NT], dt.float32)
            nc.vector.memset(onesrow[:], 1.0)
            nshift = wpool.tile([NT, 1], dt.float32)
            nc.vector.memset(nshift[:], -CRF_SHIFT)

            # logits^T [NT, t, b] fp32, and exp(logits - CRF_SHIFT)
            logits = big.tile([NT, T, BC], dt.float32)
            for n in range(NCHUNKS3):
                acc = pslg.tile([NT, NCH3], dt.float32, name="lg")
                for kc in range(4):
                    nc.tensor.matmul(acc[:], lw_sb[:, kc, :],
                                     hc_sb[:, kc, n * NCH3:(n + 1) * NCH3],
                                     start=(kc == 0), stop=(kc == 3))
                accv = acc[:].rearrange("p (t b) -> p t b", b=BC)
                nc.vector.tensor_scalar_add(
                    logits[:, n * (NCH3 // BC):(n + 1) * (NCH3 // BC), :],
                    accv, lb_sb[:])
            elog = big.tile([NT, T, BC], dt.float32)
            nc.scalar.activation(elog[:], logits[:], AF.Exp, bias=nshift[:])

            # exp-domain forward recursion, two chains of 4 sequences
            NBH = BC // 2
            shist = big.tile([1, BC, NREN + 1], dt.float32)
            nc.vector.memset(shist[:], 1.0)
            ahists = []
            for c in range(2):
                ah = big.tile([NT, T, NBH], dt.float32, name=f"ah{c}")
                nc.vector.tensor_scalar_mul(
                    ah[:, 0, :], elog[:, 0, c * NBH:(c + 1) * NBH], es_sb[:])
                ahists.append(ah)
            for t in range(1, T):
                for c in range(2):
                    ah = ahists[c]
                    bsl = slice(c * NBH, (c + 1) * NBH)
                    y = ps.tile([NT, NBH], dt.float32, name=f"y{c}")
                    nc.tensor.matmul(y[:], et_sb[:], ah[:, t - 1, :],
                                     start=True, stop=True)
                    if t % RENORM_EVERY == 0:
                        r = t // RENORM_EVERY - 1
                        ssum = ps.tile([NT, NBH], dt.float32, name="aux", bufs=1)[0:1]
                        nc.tensor.matmul(ssum[:], ones_sb[:], ah[:, t - 1, :],
                                         start=True, stop=True)
                        nc.vector.tensor_copy(shist[:, bsl, r], ssum[:])
                        rinv = sm.tile([1, NBH], dt.float32, name=f"rinv{c}")
                        nc.vector.reciprocal(rinv[:], ssum[:])
                        rb = ps.tile([NT, NBH], dt.float32, name="aux", bufs=1)
                        nc.tensor.matmul(rb[:], onesrow[:], rinv[:],
                                         start=True, stop=True)
                        u1 = sm.tile([NT, NBH], dt.float32, name=f"u1{c}")
                        nc.vector.tensor_tensor(u1[:], y[:], elog[:, t, bsl],
                                                AO.mult)
                        nc.vector.tensor_tensor(ah[:, t, :], u1[:], rb[:],
                                                AO.mult)
                    else:
                        nc.vector.tensor_tensor(ah[:, t, :], y[:],
                                                elog[:, t, bsl], AO.mult)

            # partition_b = ln(sum_j a[len_b-1, j] * e_end[j]) + sum_r ln(s_rb)
            # (host adds CRF_SHIFT * len_b back)
            alast = sm.tile([NT, BC], dt.float32)
            for c in range(2):
                bsl = slice(c * NBH, (c + 1) * NBH)
                prod = big.tile([NT, T, NBH], dt.float32, name=f"prod{c}")
                nc.vector.tensor_tensor(prod[:], ahists[c][:], ls_sb[:, :, bsl],
                                        AO.mult)
                nc.vector.reduce_sum(alast[:, bsl],
                                     prod[:].rearrange("p t b -> p b t"),
                                     axis=mybir.AxisListType.X)
            w2 = sm.tile([NT, BC], dt.float32)
            nc.vector.tensor_scalar_mul(w2[:], alast[:], ee_sb[:])
            fsum = ps.tile([1, BC], dt.float32, name="faux", bufs=1)
            nc.tensor.matmul(fsum[:], ones_sb[:], w2[:], start=True, stop=True)
            pln = sm.tile([1, BC], dt.float32)
            nc.scalar.activation(pln[:], fsum[:], AF.Ln)
            slog = sm.tile([1, BC, NREN + 1], dt.float32)
            nc.scalar.activation(slog[:], shist[:], AF.Ln)
            slogm = sm.tile([1, BC, NREN + 1], dt.float32)
            nc.vector.tensor_tensor(slogm[:], slog[:], sm_sb[:], AO.mult)
            zb = sm.tile([1, BC], dt.float32)
            nc.vector.reduce_sum(zb[:], slogm[:], axis=mybir.AxisListType.X)
            pout = sm.tile([1, BC], dt.float32)
            nc.vector.tensor_tensor(pout[:], pln[:], zb[:], AO.add)
            nc.sync.dma_start(part_out[:], pout[:])

            # emission score total
            eprod = big.tile([NT, T, BC], dt.float32)
            nc.vector.tensor_tensor(
                eprod[:], logits[:],
                em_sb[:].rearrange("p (t b) -> p t b", b=BC), AO.mult)
            erow = sm.tile([NT, 1], dt.float32)
            nc.vector.reduce_sum(erow[:], eprod[:], axis=mybir.AxisListType.XY)
            etot = ps.tile([1, 1], dt.float32, name="faux", bufs=1)
            nc.tensor.matmul(etot[:], ones_sb[:], erow[:], start=True, stop=True)
            eout = sm.tile([1, 1], dt.float32)
            nc.vector.tensor_copy(eout[:], etot[:])
            nc.sync.dma_start(emit_out[:], eout[:])
    nc.compile()
    return nc


# --------------------------------------------------------------------------
# host-side data prep
# --------------------------------------------------------------------------

def _layer_inputs(xin, w_ih, w_hh, b_ih, b_hh):
    """Per-core input dicts for one layer launch.

    xin: [2, B, T, K] fp32 (xin[1] already reversed+masked)
    w_ih: [2, 4HD, K]; w_hh: [2, 4HD, HD]; b_ih, b_hh: [2, 4HD]
    """
    K = xin.shape[-1]
    kc_in = K // 128
    # scale the g-gate rows (block 3) by 2: tanh(x) = 2*sig(2x)-1
    gscale = np.ones((4 * HD, 1), np.float32)
    gscale[2 * HD:3 * HD] = 2.0
    per_dir = []
    for d in range(2):
        wih_p = w_ih[d][_PERM] * gscale
        whh_p = w_hh[d][_PERM] * gscale
        b_p = (b_ih[d] + b_hh[d])[_PERM] * gscale[:, 0]
        wihT = np.zeros((kc_in + 1, 128, 4 * HD), np.float32)
        wihT[:kc_in] = wih_p.T.reshape(kc_in, 128, 4 * HD)
        wihT[kc_in, 0, :] = b_p          # bias plane: row 0 only
        whhT = np.ascontiguousarray(
            whh_p.T.reshape(2, 128, 4 * HD)).astype(BF16)
        per_dir.append((wihT.astype(BF16), whhT))
    maps = []
    for core in range(NCORES):
        d, q = divmod(core, 4)
        xc = xin[d, q * BL:(q + 1) * BL]              # [BL, T, K]
        xT = np.zeros((kc_in + 1, 128, T, BL), np.float32)
        xT[:kc_in] = xc.transpose(2, 1, 0).reshape(kc_in, 128, T, BL)
        xT[kc_in, 0] = 1.0               # ones plane: row 0 only
        wihT, whhT = per_dir[d]
        maps.append({"xT": np.ascontiguousarray(xT).astype(BF16),
                     "wih": wihT, "whh": whhT})
    return maps


def _collect_h(results):
    """per-core 'hout' [128,2,T,BL] bf16 -> h [2, B, T, HD] fp32."""
    h = np.empty((2, B, T, HD), np.float32)
    for core in range(NCORES):
        d, q = divmod(core, 4)
        ho = np.asarray(results[core]["hout"], dtype=np.float32)
        ho = ho.reshape(128, 2, T, BL)
        h[d, q * BL:(q + 1) * BL] = ho.transpose(3, 2, 1, 0).reshape(BL, T, HD)
    return h


def _unreverse(h_rev, lens, valid):
    """h_rev[b, s] holds position lens_b-1-s; return h[b, t] (zeros at pad)."""
    t = np.arange(T)
    idx = np.clip(lens[:, None] - 1 - t[None, :], 0, T - 1)
    out = np.take_along_axis(h_rev, idx[:, :, None], axis=1)
    return out * valid[:, :, None]


def kernel(**inputs):
    _, _, _, _, run_bass_kernel_spmd = _mods()
    global LAST_RESULTS
    LAST_RESULTS = []
    trace = bool(int(os.environ.get("KERNEL_TRACE", "0")))
    if trace:
        _install_ntff_shim()

    tokens = np.asarray(inputs["tokens"]).astype(np.int64)
    lens = np.asarray(inputs["lens"]).astype(np.int64)
    labels = np.asarray(inputs["labels"]).astype(np.int64)
    emb = np.asarray(inputs["emb"], dtype=np.float32)
    w_ih = [np.asarray(inputs["w_ih_l0"], np.float32),
            np.asarray(inputs["w_ih_l1"], np.float32)]
    w_hh = [np.asarray(inputs["w_hh_l0"], np.float32),
            np.asarray(inputs["w_hh_l1"], np.float32)]
    b_ih = [np.asarray(inputs["b_ih_l0"], np.float32),
            np.asarray(inputs["b_ih_l1"], np.float32)]
    b_hh = [np.asarray(inputs["b_hh_l0"], np.float32),
            np.asarray(inputs["b_hh_l1"], np.float32)]
    lin_w = np.asarray(inputs["lin_w"], np.float32)
    lin_b = np.asarray(inputs["lin_b"], np.float32)
    trans = np.asarray(inputs["trans"], np.float32)
    start_t = np.asarray(inputs["start_t"], np.float32)
    end_t = np.asarray(inputs["end_t"], np.float32)

    t_ar = np.arange(T)
    valid = (t_ar[None, :] < lens[:, None]).astype(np.float32)
    rev_idx = np.clip(lens[:, None] - 1 - t_ar[None, :], 0, T - 1)

    if "layer0" not in _CACHE:
        _CACHE["layer0"] = build_layer_program(E // 128)
    if "layer1" not in _CACHE:
        _CACHE["layer1"] = build_layer_program(2 * HD // 128)
    if "crf" not in _CACHE:
        _CACHE["crf"] = build_crf_program()

    cores = list(range(NCORES))

    # ---------- launch 1: layer 0 ----------
    x = emb[tokens]
    x_rev = np.take_along_axis(x, rev_idx[:, :, None], axis=1) * valid[:, :, None]
    xin0 = np.stack([x, x_rev])
    res1 = run_bass_kernel_spmd(
        _CACHE["layer0"], _layer_inputs(xin0, w_ih[0], w_hh[0], b_ih[0], b_hh[0]),
        cores, trace=trace)
    LAST_RESULTS.append(res1)
    h0 = _collect_h(res1.results)

    # ---------- launch 2: layer 1 ----------
    h0f = h0[0] * valid[:, :, None]
    h0b = _unreverse(h0[1], lens, valid)
    x1 = np.concatenate([h0f, h0b], axis=-1)
    x1_rev = np.take_along_axis(x1, rev_idx[:, :, None], axis=1) * valid[:, :, None]
    xin1 = np.stack([x1, x1_rev])
    res2 = run_bass_kernel_spmd(
        _CACHE["layer1"], _layer_inputs(xin1, w_ih[1], w_hh[1], b_ih[1], b_hh[1]),
        cores, trace=trace)
    LAST_RESULTS.append(res2)
    h1 = _collect_h(res2.results)

    # ---------- launch 3: logits + CRF ----------
    h1f = h1[0] * valid[:, :, None]
    h1b = _unreverse(h1[1], lens, valid)
    hcat = np.concatenate([h1f, h1b], axis=-1)

    lw = np.ascontiguousarray(lin_w.T.reshape(4, 128, NT)).astype(BF16)
    et = np.exp(trans).astype(np.float32)
    es = np.exp(start_t).astype(np.float32)[:, None]
    ee = np.exp(end_t).astype(np.float32)[:, None]
    lb = np.ascontiguousarray(lin_b.astype(np.float32)[:, None])
    maps = []
    for core in range(NCORES):
        bs = slice(core * BC, (core + 1) * BC)
        hc = hcat[bs]
        hcT = np.ascontiguousarray(
            hc.transpose(2, 1, 0).reshape(4, 128, T * BC)).astype(BF16)
        em = np.zeros((NT, T, BC), np.float32)
        lab = labels[bs]
        for bb in range(BC):
            em[lab[bb], np.arange(T), bb] = valid[bs][bb]
        ls = np.zeros((NT, T, BC), np.float32)
        for bb in range(BC):
            ls[:, lens[bs][bb] - 1, bb] = 1.0
        r_idx = np.arange(NREN + 1)
        smk = (RENORM_EVERY * (r_idx[None] + 1)
               <= (lens[bs] - 1)[:, None]).astype(np.float32)[None]
        maps.append({
            "hcat": hcT, "linw": lw, "linb": lb, "etrans": et,
            "estart": es, "eend": ee,
            "emitmask": np.ascontiguousarray(
                em.reshape(NT, T * BC)).astype(BF16),
            "lastsel": np.ascontiguousarray(ls).astype(BF16),
            "smask": np.ascontiguousarray(smk),
        })
    res3 = run_bass_kernel_spmd(_CACHE["crf"], maps, cores, trace=trace)
    LAST_RESULTS.append(res3)

    partition = np.concatenate(
        [np.asarray(r["part_out"])[0] for r in res3.results])
    # undo the exp(-CRF_SHIFT) per-step shift: a'_t = a_t * e^{-shift*(t+1)}
    partition = partition + CRF_SHIFT * lens.astype(np.float32)
    emit = float(sum(np.asarray(r["emit_out"])[0, 0] for r in res3.results))

    # host-side numerator terms
    first_tag = labels[:, 0]
    last_tag = np.take_along_axis(labels, (lens - 1)[:, None], axis=1)[:, 0]
    tr_sc = float((trans[labels[:, :-1], labels[:, 1:]] * valid[:, 1:]).sum())
    host_num = float(start_t[first_tag].sum()) + tr_sc + float(end_t[last_tag].sum())

    loss = partition.sum() - emit - host_num
    return np.float32(loss)
